# revision 23
# baseline (speedup 1.0000x reference)
"""Trainium2 Bass kernel for nn_Channel_attention (B=4, D=4, H=32, W=32, C=64).

Computation (per batch b, with X = x[b].reshape(N=4096, C=64)):
    S   = X @ X.T                      [N, N]
    P   = softmax(S, axis=-1)
    Y   = P @ X                        [N, C]
    G   = Y * X                        elementwise gate
    out = relu(conv3d_114(G) + bias)   [D, H, W-3, 2C]

Sharding: 8 cores = (batch b in 0..3) x (half of the N=4096 tokens).
Each core computes attention for its 2048 query tokens against all 4096
keys of its batch, then the gate and the (1,1,4)-conv for those tokens
(the conv only spans W, so a split at a D boundary is conv-local).
The host rolls each core's token axis so its queries sit at positions
0..2048; softmax over keys is permutation invariant.

Device decomposition per core (q = 2048 queries, k = 4096 keys):
  MM1 (PE, fp16):  S^T tile [k=128, q=512] = (X^T[:,kc])^T @ X^T[:,qt]
                   contraction C=64 -> two k-chunks row-packed into PE
                   rows 0-63 / 64-127 (xt input holds X^T twice).
  exp (ACT):       E^T = exp(S^T - 64) from PSUM -> bf16 SBUF. The bias
                   keeps exp in range; it cancels in the normalization.
  MM2 (PE, bf16):  U^T [65, q] += xe[kc]^T @ E^T over all kc, where
                   xe = [X | ones]: row 64 of U^T is the softmax
                   denominator for free. xe is split hi+lo bf16 so the
                   accumulation carries ~fp32 precision (MM2_LO).
  normalize+gate:  r = 1/U^T[64]; broadcast across partitions with a
                   K=1 matmul; G^T = U^T[0:64] * r * X^T (f32) -> fp16.
  conv (PE, fp16): out[n, o] = sum_t G^T[:, n+t]^T @ W[t] (+ bias via
                   K=1 matmul), relu on DVE, valid w<29 cols DMA'd out.

The emission is software-pipelined: MM2 of pair g-1 is emitted after
MM1/exp of pair g so the in-order PE stream always has independent work
while ACT computes the exp it needs next; the normalization broadcast
matmul is deferred several pairs so the (slow, exact) DVE reciprocal is
finished before PE reaches it.
"""

import numpy as np
import ml_dtypes

B, D, H, W, C = 4, 4, 32, 32, 64
N = D * H * W          # 4096 tokens per batch
NQ = N // 2            # 2048 queries per core
OC = 2 * C             # 128 conv output channels
WO = W - 3             # 29 valid conv outputs per (d, h) row
QT = 512               # query tile (psum bank / fp32 moving-dim limit)
NKC = N // 128         # 32 key chunks of 128
NQT = NQ // QT         # 4 query tiles per core
NPAIR = NKC // 2       # 16 key-chunk pairs per query tile
EXP_BIAS = 64.0        # exp(s - 64): keeps exp finite for s in [-46, 106]
MM2_LO = True          # hi+lo bf16 split for the E @ X matmul
NORM_B_LAG = 10         # pairs between last MM2 of a qtile and its rb matmul
CONV_LAG = 2           # further pairs before that qtile's conv subtiles

_CACHE = {}


def _build_nc(debug=False):
    import concourse.bacc as bacc
    import concourse.bass as bass
    import concourse.tile as tile
    from concourse import mybir

    f32 = mybir.dt.float32
    f16 = mybir.dt.float16
    bf16 = mybir.dt.bfloat16

    nc = bacc.Bacc("TRN2", target_bir_lowering=False, debug=False,
                   num_devices=8)

    xt_d = nc.dram_tensor("xt", [128, N], f16, kind="ExternalInput").ap()
    xq_d = nc.dram_tensor("xq", [C, NQ], f32, kind="ExternalInput").ap()
    xeh_d = nc.dram_tensor("xe_hi", [128, NKC, C + 1], bf16,
                           kind="ExternalInput").ap()
    xel_d = nc.dram_tensor("xe_lo", [128, NKC, C + 1], bf16,
                           kind="ExternalInput").ap()
    wc_d = nc.dram_tensor("wc", [C + 1, 4, OC], f16,
                          kind="ExternalInput").ap()
    out_d = nc.dram_tensor("out", [2 * H * W, OC], f32,
                           kind="ExternalOutput").ap()
    if debug:
        dbg_g_d = nc.dram_tensor("dbg_g", [C, NQ + 8], f16,
                                 kind="ExternalOutput").ap()
        dbg_u_d = nc.dram_tensor("dbg_u", [C + 1, NQ], f32,
                                 kind="ExternalOutput").ap()

    GPAD = 8  # zero columns after the 2048 gated queries (conv overrun)

    with tile.TileContext(nc) as tc:
        with (
            tc.tile_pool(name="sb_in", bufs=1) as sb_in,
            tc.tile_pool(name="sb_e", bufs=3) as sb_e,
            tc.tile_pool(name="sb_g", bufs=1) as sb_g,
            tc.tile_pool(name="sb_r", bufs=2) as sb_r,
            tc.tile_pool(name="sb_t", bufs=2) as sb_t,
            tc.tile_pool(name="sb_o", bufs=3) as sb_o,
            tc.tile_pool(name="ps_s", bufs=2, space="PSUM") as ps_s,
            tc.tile_pool(name="ps_y", bufs=2, space="PSUM") as ps_y,
            tc.tile_pool(name="ps_a", bufs=2, space="PSUM") as ps_a,
        ):
            # ---- input loads, ordered so pair-0 deps land first ---------
            xt = [sb_in.tile([128, 1024], f16, tag=f"xt{m}", name=f"xt{m}")
                  for m in range(4)]
            xeh = [sb_in.tile([128, 8, C + 1], bf16, tag=f"xeh{m}",
                              name=f"xeh{m}") for m in range(4)]
            xel = [sb_in.tile([128, 8, C + 1], bf16, tag=f"xel{m}",
                              name=f"xel{m}") for m in range(4)]
            nc.sync.dma_start(xt[0][:, 0:512], xt_d[:, 0:512])
            nc.sync.dma_start(xt[0][:, 512:1024], xt_d[:, 512:1024])
            nc.gpsimd.dma_start(xeh[0], xeh_d[:, 0:8, :])
            nc.gpsimd.dma_start(xel[0], xel_d[:, 0:8, :])
            for m in range(1, 4):
                nc.sync.dma_start(xt[m], xt_d[:, 1024 * m:1024 * (m + 1)])
                nc.gpsimd.dma_start(xeh[m], xeh_d[:, 8 * m:8 * (m + 1), :])
                nc.gpsimd.dma_start(xel[m], xel_d[:, 8 * m:8 * (m + 1), :])
            xq = sb_in.tile([C, NQ], f32, tag="xq")
            nc.sync.dma_start(xq, xq_d)
            wc = sb_in.tile([C + 1, 4, OC], f16, tag="wc")
            nc.sync.dma_start(wc, wc_d)

            nbias = sb_in.tile([128, 1], f32, tag="nbias")
            nc.vector.memset(nbias, -EXP_BIAS)

            ones32 = sb_in.tile([65, C], f32, tag="ones32")
            nc.vector.memset(ones32, 1.0)

            gT = sb_g.tile([C + 1, NQ + GPAD], f16, tag="gT")
            nc.vector.memset(gT[0:C, NQ:], 0.0)
            nc.vector.memset(gT[C:C + 1, :], 1.0)

            psY = [None] * NQT
            esb = [None] * (NQT * NPAIR)
            rtile = [None] * NQT

            def emit_mm1_exp(g):
                """Pair g: two row-packed fp16 score MMs + one exp."""
                j, p = g // NPAIR, g % NPAIR
                if p == 0:
                    psY[j] = ps_y.tile([C + 1, QT], f32, tag="psY",
                                       name="psY")
                mq = (QT * j) // 1024
                qloc = (QT * j) % 1024
                kc0, kc1 = 2 * p, 2 * p + 1
                m0, c0 = kc0 // 8, (kc0 % 8) * 128
                m1, c1 = kc1 // 8, (kc1 % 8) * 128
                st = ps_s.tile([128, 1024], f32, tag="st", name="st")
                nc.tensor.matmul(st[:, 0:QT],
                                 xt[m0][0:C, c0:c0 + 128],
                                 xt[mq][0:C, qloc:qloc + QT],
                                 start=True, stop=True)
                nc.tensor.matmul(st[:, QT:1024],
                                 xt[m1][C:128, c1:c1 + 128],
                                 xt[mq][C:128, qloc:qloc + QT],
                                 start=True, stop=True)
                e = sb_e.tile([128, 1024], mybir.dt.bfloat16, tag="e",
                              name="e")
                nc.scalar.activation(e, st,
                                     mybir.ActivationFunctionType.Exp,
                                     bias=nbias[:, 0:1], scale=1.0)
                esb[g] = e

            def emit_mm2(g):
                """Accumulate U^T += xe^T @ E^T for both chunks of pair g."""
                j, p = g // NPAIR, g % NPAIR
                e = esb[g]
                for half, kc in ((0, 2 * p), (1, 2 * p + 1)):
                    m, s8 = kc // 8, kc % 8
                    er = e[:, QT * half:QT * (half + 1)]
                    first = p == 0 and half == 0
                    last = p == NPAIR - 1 and half == 1
                    # lo-correction only where softmax mass lives: the
                    # diagonal chunks (keys == this qtile's queries).
                    # Off-diagonal softmax mass is <=1e-4, so its lo term
                    # is ~1e-7 relative - dropped.
                    lo = MM2_LO and p in (2 * j, 2 * j + 1)
                    nc.tensor.matmul(psY[j], xeh[m][:, s8, :], er,
                                     start=first, stop=last and not lo)
                    if lo:
                        nc.tensor.matmul(psY[j], xel[m][:, s8, :], er,
                                         start=False, stop=last)

            def emit_norm_a(j):
                """DVE part of the normalization: r = 1/sum."""
                r = sb_r.tile([65, QT], f32, tag="r", name="r")
                nc.vector.reciprocal(r[64:65, :], psY[j][64:65, :])
                rtile[j] = r

            def emit_norm_b(j):
                """Broadcast r across partitions; gate into G^T (fp16)."""
                pY = psY[j]
                r = rtile[j]
                rb = ps_a.tile([128, QT], f32, tag="cp", name="rb")
                nc.tensor.matmul(rb[0:C, :], ones32[64:65, :], r[64:65, :],
                                 start=True, stop=True)
                rbf = rb[0:C, :]
                q0 = QT * j
                if debug:
                    ustage = sb_t.tile([C + 1, QT], f32, tag="ustage",
                                       name="ustage")
                    nc.vector.tensor_copy(ustage, pY)
                    nc.sync.dma_start(dbg_u_d[:, q0:q0 + QT], ustage)
                tmp = sb_t.tile([C, QT], f32, tag="tmp", name="tmp")
                nc.vector.tensor_mul(tmp, xq[:, q0:q0 + QT], rbf)
                nc.vector.tensor_mul(gT[0:C, q0:q0 + QT], tmp, pY[0:C, :])

            def emit_conv_sub(i):
                """Conv subtile i: 128 output positions [128i, 128i+128)."""
                base = 128 * i
                cp = ps_a.tile([128, OC], f32, tag="cp", name="cp")
                for t in range(4):
                    nc.tensor.matmul(cp,
                                     gT[:, base + t:base + t + 128],
                                     wc[:, t, :], start=(t == 0),
                                     stop=(t == 3))
                ot = sb_o.tile([128, OC], f32, tag="ot", name="ot")
                nc.vector.tensor_scalar_max(ot, cp, 0.0)
                eng = nc.sync if i % 2 == 0 else nc.gpsimd
                eng.dma_start(out_d[128 * i:128 * (i + 1), :], ot)


            # ---- software-pipelined emission ----------------------------
            # conv subtiles are spread one-per-pair to avoid PE bursts;
            # subtiles 8..10 are held back as PE filler for the tail
            # reciprocal, 11..15 need the final gate.
            from collections import deque
            pending = deque()
            NG = NQT * NPAIR  # 64 pairs
            for g in range(NG + 1):
                if g < NG:
                    emit_mm1_exp(g)
                if g > 0:
                    gm = g - 1
                    emit_mm2(gm)
                    if gm % NPAIR == NPAIR - 1:
                        emit_norm_a(gm // NPAIR)
                if g >= NORM_B_LAG and (g - NORM_B_LAG) % NPAIR == NPAIR - 1:
                    jj = (g - NORM_B_LAG) // NPAIR
                    emit_norm_b(jj)
                    pending.extend({0: [0, 1, 2], 1: [3, 4, 5, 6],
                                    2: [7, 8]}.get(jj, []))
                elif pending and g % NPAIR in (0, 1, 2) and g >= NPAIR:
                    # pop conv work right after a qtile boundary: it is the
                    # window where PE otherwise stalls on the reciprocal
                    emit_conv_sub(pending.popleft())
            while pending:  # PE filler while the tail recip runs
                emit_conv_sub(pending.popleft())
            for i in (9, 10):
                emit_conv_sub(i)
            emit_norm_b(NQT - 1)
            if debug:
                nc.sync.dma_start(dbg_g_d, gT)
            for i in (11, 12, 13, 14, 15):
                emit_conv_sub(i)

    nc.compile()
    return nc


def _get_nc(debug=False):
    key = ("nc", debug, MM2_LO, NORM_B_LAG, CONV_LAG)
    if key not in _CACHE:
        _CACHE[key] = _build_nc(debug)
    return _CACHE[key]


def _prep_core(x, conv_w, conv_b, b_i, half):
    bf = ml_dtypes.bfloat16
    X = np.asarray(x[b_i], np.float32).reshape(N, C)
    Xr = np.roll(X, -half * NQ, axis=0)        # this core's queries first
    xt = Xr.T                                  # [64, 4096]
    xt_dup = np.concatenate([xt, xt], 0).astype(np.float16)
    xq = np.ascontiguousarray(xt[:, :NQ]).astype(np.float32)
    xe = np.concatenate([Xr, np.ones((N, 1), np.float32)], 1)  # [4096, 65]
    xe_hi = xe.astype(bf)
    xe_lo = (xe - xe_hi.astype(np.float32)).astype(bf)

    def blk(a):  # [4096, 65] -> [128, 32, 65]: chunk kc at [:, kc, :]
        return np.ascontiguousarray(
            a.reshape(NKC, 128, C + 1).transpose(1, 0, 2))

    wct = np.asarray(conv_w, np.float32)[0, 0].transpose(1, 0, 2)  # [64,4,128]
    brow = np.broadcast_to(
        np.asarray(conv_b, np.float32).reshape(1, 1, OC) / 4.0, (1, 4, OC))
    wc = np.ascontiguousarray(
        np.concatenate([wct, brow], axis=0)).astype(np.float16)  # [65,4,128]
    return {"xt": xt_dup, "xq": xq, "xe_hi": blk(xe_hi), "xe_lo": blk(xe_lo),
            "wc": wc}


def _run(x, conv_w, conv_b, trace=False, debug=False):
    from concourse import bass_utils

    nc = _get_nc(debug)
    in_maps = [_prep_core(x, conv_w, conv_b, core // 2, core % 2)
               for core in range(8)]
    res = bass_utils.run_bass_kernel_spmd(nc, in_maps,
                                          core_ids=list(range(8)),
                                          trace=trace)
    out = np.zeros((B, D, H, WO, OC), np.float32)
    for core in range(8):
        b_i, half = core // 2, core % 2
        oc = res.results[core]["out"].reshape(2, H, W, OC)
        out[b_i, 2 * half:2 * half + 2] = oc[:, :, :WO, :]
    return out, res


def kernel(x, conv_w, conv_b):
    out, _ = _run(x, conv_w, conv_b, trace=False)
    return out


# revision 24
# speedup vs baseline: 1.0873x; 1.0873x over previous
"""Trainium2 Bass kernel for nn_Channel_attention (B=4, D=4, H=32, W=32, C=64).

Computation (per batch b, with X = x[b].reshape(N=4096, C=64)):
    S   = X @ X.T                      [N, N]
    P   = softmax(S, axis=-1)
    Y   = P @ X                        [N, C]
    G   = Y * X                        elementwise gate
    out = relu(conv3d_114(G) + bias)   [D, H, W-3, 2C]

Sharding: 8 cores = (batch b in 0..3) x (half of the N=4096 tokens).
Each core computes attention for its 2048 query tokens against all 4096
keys of its batch, then the gate and the (1,1,4)-conv for those tokens
(the conv only spans W, so a split at a D boundary is conv-local).
The host rolls each core's token axis so its queries sit at positions
0..2048; softmax over keys is permutation invariant.

Device decomposition per core (q = 2048 queries, k = 4096 keys):
  MM1 (PE, fp16):  S^T tile [k=128, q=512] = (X^T[:,kc])^T @ X^T[:,qt]
                   contraction C=64 -> two k-chunks row-packed into PE
                   rows 0-63 / 64-127 (xt input holds X^T twice).
  exp (ACT):       E^T = exp(S^T - 64) from PSUM -> bf16 SBUF. The bias
                   keeps exp in range; it cancels in the normalization.
  MM2 (PE, bf16):  U^T [65, q] += xe[kc]^T @ E^T over all kc, where
                   xe = [X | ones]: row 64 of U^T is the softmax
                   denominator for free. xe is split hi+lo bf16 so the
                   accumulation carries ~fp32 precision (MM2_LO).
  normalize+gate:  r = 1/U^T[64]; broadcast across partitions with a
                   K=1 matmul; G^T = U^T[0:64] * r * X^T (f32) -> fp16.
  conv (PE, fp16): out[n, o] = sum_t G^T[:, n+t]^T @ W[t] (+ bias via
                   K=1 matmul), relu on DVE, valid w<29 cols DMA'd out.

The emission is software-pipelined: MM2 of pair g-1 is emitted after
MM1/exp of pair g so the in-order PE stream always has independent work
while ACT computes the exp it needs next; the normalization broadcast
matmul is deferred several pairs so the (slow, exact) DVE reciprocal is
finished before PE reaches it.
"""

import numpy as np
import ml_dtypes

B, D, H, W, C = 4, 4, 32, 32, 64
N = D * H * W          # 4096 tokens per batch
NQ = N // 2            # 2048 queries per core
OC = 2 * C             # 128 conv output channels
WO = W - 3             # 29 valid conv outputs per (d, h) row
QT = 512               # query tile (psum bank / fp32 moving-dim limit)
NKC = N // 128         # 32 key chunks of 128
NQT = NQ // QT         # 4 query tiles per core
NPAIR = NKC // 2       # 16 key-chunk pairs per query tile
EXP_BIAS = 64.0        # exp(s - 64): keeps exp finite for s in [-46, 106]
MM2_LO = True          # hi+lo bf16 split for the E @ X matmul
NORM_B_LAG = 10         # pairs between last MM2 of a qtile and its rb matmul
CONV_LAG = 2           # further pairs before that qtile's conv subtiles

_CACHE = {}


def _build_nc(debug=False):
    import concourse.bacc as bacc
    import concourse.bass as bass
    import concourse.tile as tile
    from concourse import mybir

    f32 = mybir.dt.float32
    f16 = mybir.dt.float16
    bf16 = mybir.dt.bfloat16

    nc = bacc.Bacc("TRN2", target_bir_lowering=False, debug=False,
                   num_devices=8)

    xt_d = nc.dram_tensor("xt", [128, N], f16, kind="ExternalInput").ap()
    xq_d = nc.dram_tensor("xq", [C, NQ], f32, kind="ExternalInput").ap()
    xeh_d = nc.dram_tensor("xe_hi", [128, NKC, C + 1], bf16,
                           kind="ExternalInput").ap()
    xel_d = nc.dram_tensor("xe_lo", [128, NKC, C + 1], bf16,
                           kind="ExternalInput").ap()
    wc_d = nc.dram_tensor("wc", [C + 1, 4, OC], f16,
                          kind="ExternalInput").ap()
    out_d = nc.dram_tensor("out", [2 * H * W, OC], f32,
                           kind="ExternalOutput").ap()
    if debug:
        dbg_g_d = nc.dram_tensor("dbg_g", [C, NQ + 8], f16,
                                 kind="ExternalOutput").ap()
        dbg_u_d = nc.dram_tensor("dbg_u", [C + 1, NQ], f32,
                                 kind="ExternalOutput").ap()

    GPAD = 8  # zero columns after the 2048 gated queries (conv overrun)

    with tile.TileContext(nc) as tc:
        with (
            tc.tile_pool(name="sb_in", bufs=1) as sb_in,
            tc.tile_pool(name="sb_e", bufs=3) as sb_e,
            tc.tile_pool(name="sb_g", bufs=1) as sb_g,
            tc.tile_pool(name="sb_r", bufs=2) as sb_r,
            tc.tile_pool(name="sb_t", bufs=2) as sb_t,
            tc.tile_pool(name="sb_o", bufs=3) as sb_o,
            tc.tile_pool(name="ps_s", bufs=2, space="PSUM") as ps_s,
            tc.tile_pool(name="ps_y", bufs=2, space="PSUM") as ps_y,
            tc.tile_pool(name="ps_a", bufs=2, space="PSUM") as ps_a,
        ):
            # ---- input loads, ordered so pair-0 deps land first ---------
            xt = [sb_in.tile([128, 1024], f16, tag=f"xt{m}", name=f"xt{m}")
                  for m in range(4)]
            xeh = [sb_in.tile([128, 8, C + 1], bf16, tag=f"xeh{m}",
                              name=f"xeh{m}") for m in range(4)]
            xel = [sb_in.tile([128, 8, C + 1], bf16, tag=f"xel{m}",
                              name=f"xel{m}") for m in range(4)]
            nc.sync.dma_start(xt[0][:, 0:512], xt_d[:, 0:512])
            nc.sync.dma_start(xt[0][:, 512:1024], xt_d[:, 512:1024])
            nc.gpsimd.dma_start(xeh[0], xeh_d[:, 0:8, :])
            nc.gpsimd.dma_start(xel[0], xel_d[:, 0:8, :])
            for m in range(1, 4):
                nc.sync.dma_start(xt[m], xt_d[:, 1024 * m:1024 * (m + 1)])
                nc.gpsimd.dma_start(xeh[m], xeh_d[:, 8 * m:8 * (m + 1), :])
                nc.gpsimd.dma_start(xel[m], xel_d[:, 8 * m:8 * (m + 1), :])
            xq = sb_in.tile([C, NQ], f32, tag="xq")
            nc.sync.dma_start(xq, xq_d)
            wc = sb_in.tile([C + 1, 4, OC], f16, tag="wc")
            nc.sync.dma_start(wc, wc_d)

            nbias = sb_in.tile([128, 1], f32, tag="nbias")
            nc.vector.memset(nbias, -EXP_BIAS)

            ones32 = sb_in.tile([65, C], f32, tag="ones32")
            nc.vector.memset(ones32, 1.0)

            gT = sb_g.tile([C + 1, NQ + GPAD], f16, tag="gT")
            nc.vector.memset(gT[0:C, NQ:], 0.0)
            nc.vector.memset(gT[C:C + 1, :], 1.0)

            psY = [None] * NQT
            esb = [None] * (NQT * NPAIR)
            rtile = [None] * NQT

            def emit_mm1_exp(g):
                """Pair g: two row-packed fp16 score MMs + one exp."""
                j, p = g // NPAIR, g % NPAIR
                if p == 0:
                    psY[j] = ps_y.tile([C + 1, QT], f32, tag="psY",
                                       name="psY")
                mq = (QT * j) // 1024
                qloc = (QT * j) % 1024
                kc0, kc1 = 2 * p, 2 * p + 1
                m0, c0 = kc0 // 8, (kc0 % 8) * 128
                m1, c1 = kc1 // 8, (kc1 % 8) * 128
                st = ps_s.tile([128, 1024], f32, tag="st", name="st")
                nc.tensor.matmul(st[:, 0:QT],
                                 xt[m0][0:C, c0:c0 + 128],
                                 xt[mq][0:C, qloc:qloc + QT],
                                 start=True, stop=True)
                nc.tensor.matmul(st[:, QT:1024],
                                 xt[m1][C:128, c1:c1 + 128],
                                 xt[mq][C:128, qloc:qloc + QT],
                                 start=True, stop=True)
                e = sb_e.tile([128, 1024], mybir.dt.bfloat16, tag="e",
                              name="e")
                nc.scalar.activation(e, st,
                                     mybir.ActivationFunctionType.Exp,
                                     bias=nbias[:, 0:1], scale=1.0)
                esb[g] = e

            def emit_mm2(g):
                """Accumulate U^T += xe^T @ E^T for both chunks of pair g."""
                j, p = g // NPAIR, g % NPAIR
                e = esb[g]
                for half, kc in ((0, 2 * p), (1, 2 * p + 1)):
                    m, s8 = kc // 8, kc % 8
                    er = e[:, QT * half:QT * (half + 1)]
                    first = p == 0 and half == 0
                    last = p == NPAIR - 1 and half == 1
                    # lo-correction only where softmax mass lives: the
                    # diagonal chunks (keys == this qtile's queries).
                    # Off-diagonal softmax mass is <=1e-4, so its lo term
                    # is ~1e-7 relative - dropped.
                    lo = MM2_LO and p in (2 * j, 2 * j + 1)
                    nc.tensor.matmul(psY[j], xeh[m][:, s8, :], er,
                                     start=first, stop=last and not lo)
                    if lo:
                        nc.tensor.matmul(psY[j], xel[m][:, s8, :], er,
                                         start=False, stop=last)

            def emit_norm_a(j):
                """r = 1/sum on DVE, in halves so rb can start sooner."""
                r = sb_r.tile([65, QT], f32, tag="r", name="r")
                hq = QT // 2
                nc.vector.reciprocal(r[64:65, 0:hq], psY[j][64:65, 0:hq])
                nc.vector.reciprocal(r[64:65, hq:QT], psY[j][64:65, hq:QT])
                rtile[j] = r

            def emit_norm_b(j):
                """Broadcast r across partitions; gate into G^T (fp16)."""
                pY = psY[j]
                r = rtile[j]
                rb = ps_a.tile([128, QT], f32, tag="cp", name="rb")
                hq = QT // 2
                nc.tensor.matmul(rb[0:C, 0:hq], ones32[64:65, :],
                                 r[64:65, 0:hq], start=True, stop=True)
                nc.tensor.matmul(rb[0:C, hq:QT], ones32[64:65, :],
                                 r[64:65, hq:QT], start=True, stop=True)
                rbf = rb[0:C, :]
                q0 = QT * j
                if debug:
                    ustage = sb_t.tile([C + 1, QT], f32, tag="ustage",
                                       name="ustage")
                    nc.vector.tensor_copy(ustage, pY)
                    nc.sync.dma_start(dbg_u_d[:, q0:q0 + QT], ustage)
                tmp = sb_t.tile([C, QT], f32, tag="tmp", name="tmp")
                nc.vector.tensor_mul(tmp[:, 0:hq], xq[:, q0:q0 + hq],
                                     rbf[:, 0:hq])
                nc.vector.tensor_mul(gT[0:C, q0:q0 + hq], tmp[:, 0:hq],
                                     pY[0:C, 0:hq])
                nc.vector.tensor_mul(tmp[:, hq:QT], xq[:, q0 + hq:q0 + QT],
                                     rbf[:, hq:QT])
                nc.vector.tensor_mul(gT[0:C, q0 + hq:q0 + QT], tmp[:, hq:QT],
                                     pY[0:C, hq:QT])

            def emit_conv_sub(i):
                """Conv subtile i: 128 output positions [128i, 128i+128)."""
                base = 128 * i
                cp = ps_a.tile([128, OC], f32, tag="cp", name="cp")
                for t in range(4):
                    nc.tensor.matmul(cp,
                                     gT[:, base + t:base + t + 128],
                                     wc[:, t, :], start=(t == 0),
                                     stop=(t == 3))
                ot = sb_o.tile([128, OC], f32, tag="ot", name="ot")
                nc.vector.tensor_scalar_max(ot, cp, 0.0)
                eng = nc.sync if i % 2 == 0 else nc.gpsimd
                eng.dma_start(out_d[128 * i:128 * (i + 1), :], ot)


            # ---- software-pipelined emission ----------------------------
            # conv subtiles are spread one-per-pair to avoid PE bursts;
            # subtiles 8..10 are held back as PE filler for the tail
            # reciprocal, 11..15 need the final gate.
            from collections import deque
            pending = deque()
            NG = NQT * NPAIR  # 64 pairs
            for g in range(NG + 1):
                if g < NG:
                    emit_mm1_exp(g)
                if g > 0:
                    gm = g - 1
                    emit_mm2(gm)
                    if gm % NPAIR == NPAIR - 1:
                        emit_norm_a(gm // NPAIR)
                if g >= NORM_B_LAG and (g - NORM_B_LAG) % NPAIR == NPAIR - 1:
                    jj = (g - NORM_B_LAG) // NPAIR
                    emit_norm_b(jj)
                    pending.extend({0: [0, 1, 2], 1: [3, 4, 5, 6],
                                    2: [7, 8]}.get(jj, []))
                elif pending and g % NPAIR in (0, 1, 2) and g >= NPAIR:
                    # pop conv work right after a qtile boundary: it is the
                    # window where PE otherwise stalls on the reciprocal
                    emit_conv_sub(pending.popleft())
            while pending:  # PE filler while the tail recip runs
                emit_conv_sub(pending.popleft())
            for i in (9, 10):
                emit_conv_sub(i)
            emit_norm_b(NQT - 1)
            if debug:
                nc.sync.dma_start(dbg_g_d, gT)
            for i in (11, 12, 13, 14, 15):
                emit_conv_sub(i)

    nc.compile()
    return nc


def _get_nc(debug=False):
    key = ("nc", debug, MM2_LO, NORM_B_LAG, CONV_LAG)
    if key not in _CACHE:
        _CACHE[key] = _build_nc(debug)
    return _CACHE[key]


def _prep_core(x, conv_w, conv_b, b_i, half):
    bf = ml_dtypes.bfloat16
    X = np.asarray(x[b_i], np.float32).reshape(N, C)
    Xr = np.roll(X, -half * NQ, axis=0)        # this core's queries first
    xt = Xr.T                                  # [64, 4096]
    xt_dup = np.concatenate([xt, xt], 0).astype(np.float16)
    xq = np.ascontiguousarray(xt[:, :NQ]).astype(np.float32)
    xe = np.concatenate([Xr, np.ones((N, 1), np.float32)], 1)  # [4096, 65]
    xe_hi = xe.astype(bf)
    xe_lo = (xe - xe_hi.astype(np.float32)).astype(bf)

    def blk(a):  # [4096, 65] -> [128, 32, 65]: chunk kc at [:, kc, :]
        return np.ascontiguousarray(
            a.reshape(NKC, 128, C + 1).transpose(1, 0, 2))

    wct = np.asarray(conv_w, np.float32)[0, 0].transpose(1, 0, 2)  # [64,4,128]
    brow = np.broadcast_to(
        np.asarray(conv_b, np.float32).reshape(1, 1, OC) / 4.0, (1, 4, OC))
    wc = np.ascontiguousarray(
        np.concatenate([wct, brow], axis=0)).astype(np.float16)  # [65,4,128]
    return {"xt": xt_dup, "xq": xq, "xe_hi": blk(xe_hi), "xe_lo": blk(xe_lo),
            "wc": wc}


def _run(x, conv_w, conv_b, trace=False, debug=False):
    from concourse import bass_utils

    nc = _get_nc(debug)
    in_maps = [_prep_core(x, conv_w, conv_b, core // 2, core % 2)
               for core in range(8)]
    res = bass_utils.run_bass_kernel_spmd(nc, in_maps,
                                          core_ids=list(range(8)),
                                          trace=trace)
    out = np.zeros((B, D, H, WO, OC), np.float32)
    for core in range(8):
        b_i, half = core // 2, core % 2
        oc = res.results[core]["out"].reshape(2, H, W, OC)
        out[b_i, 2 * half:2 * half + 2] = oc[:, :, :WO, :]
    return out, res


def kernel(x, conv_w, conv_b):
    out, _ = _run(x, conv_w, conv_b, trace=False)
    return out
